# revision 5
# baseline (speedup 1.0000x reference)
"""Trainium2 Bass kernel for nn_MultiHeadAttention (dense transformer prefill,
GQA 32q/8kv heads, RoPE, causal mask), tensor-parallel over heads across 8
NeuronCores with an AllToAll reshard before the output projection.

Self-contained: hardcodes all shapes from the problem spec.

Per-core plan (core c):
  - inputs: x^T (replicated, bf16), per-core head shards of wq/wk/wv
    (pre-transposed, RoPE pair-split permuted), full wo^T (replicated),
    cos/sin tables, mask plan.
  - QKV projections produce Q^T/K^T (head-dim on partitions) and V (token on
    partitions) via PE matmuls; RoPE applied on DVE with a half-swap trick.
  - attention computed transposed: S^T[k,q] tiles -> exp on ACT (no max
    subtraction; inputs are bounded randn) -> P^T bf16 -> AV gives O^T
    (head-dim x q) and a ones-matmul gives the softmax denominator (1 x q);
    normalization via DVE with a DRAM-roundtrip partition broadcast.
  - merged^T tiles are DMA'd into an AllToAll buffer; the collective reshards
    from head-parallel to token-parallel.
  - output projection computes this core's 512-token shard of the output
    against the full wo^T.
Host gathers the 8 token shards and concatenates.
"""

import math

import numpy as np
import ml_dtypes

import concourse.bass as bass
import concourse.tile as tile
from concourse import bacc, mybir
from concourse.bass_utils import run_bass_kernel_spmd

BF16 = ml_dtypes.bfloat16

# ---- problem constants ----
B = 2
S = 2048
D = 4096
N_HEADS = 32
N_KV = 8
HD = 128
NCORES = 8
T = B * S                      # 4096 flattened tokens
HLOC = N_HEADS // NCORES       # 4 local q heads
E = HLOC * HD                  # 512 local q dim
TSH = T // NCORES              # 512 token shard
DCH = D // 128                 # 32 contraction chunks
KC = S // 128                  # 16 k-chunks per sequence
KG = KC // 2                   # 8 k-groups (2 chunks) per sequence
QT = S // 512                  # 4 q-tiles per sequence
SCALE = 1.0 / math.sqrt(HD)

SKIP, PLAIN, MIXED_CAUSAL, MIXED_DATA = 0, 1, 2, 3


def _classify_mask(mask):
    """Classify (k-group, q-tile) tiles of mask^T. Returns (plan, mtiles).

    plan[qt] = list of (kg, mode, mtile_idx); mtiles: (n, 128, 2, 512) f32."""
    m = np.asarray(mask, np.float32).reshape(S, S)  # [q, k]
    mt = np.ascontiguousarray(m.T)  # [k, q]
    kk = np.arange(S)
    qq = np.arange(S)
    plan = []
    uniq = {}
    mtiles = []
    for qt in range(QT):
        row = []
        qs = slice(qt * 512, (qt + 1) * 512)
        for kg in range(KG):
            ks = slice(kg * 256, (kg + 1) * 256)
            sub = mt[ks, qs]
            if np.all(sub <= -1e8):
                continue  # fully masked -> skip the whole k-group
            if np.all(sub == 0.0):
                row.append((kg, PLAIN, -1))
                continue
            # mixed: exact-causal check (0 where q>=k, <=-1e8 where q<k)
            causal = qq[None, qs] >= kk[ks, None]
            if np.all((sub == 0.0) == causal) and np.all(sub[~causal] <= -1e8):
                row.append((kg, MIXED_CAUSAL, -1))
            else:
                key = sub.tobytes()
                if key not in uniq:
                    uniq[key] = len(mtiles)
                    mtiles.append(sub.reshape(2, 128, 512).transpose(1, 0, 2))
                row.append((kg, MIXED_DATA, uniq[key]))
        plan.append(row)
    if mtiles:
        mtiles = np.ascontiguousarray(np.stack(mtiles), np.float32)
    else:
        mtiles = np.zeros((1, 128, 2, 512), np.float32)
    return plan, mtiles


def _pair_split_perm(nheads):
    """Row permutation putting each head's even components first, odds second."""
    idx = []
    for h in range(nheads):
        base = h * HD
        idx.extend(base + np.arange(0, HD, 2))
        idx.extend(base + np.arange(1, HD, 2))
    return np.asarray(idx)


def _build(plan, n_mtiles, no_collective=False):
    nc = bacc.Bacc("TRN2", target_bir_lowering=False, debug=False,
                   num_devices=NCORES)
    f32, bf = mybir.dt.float32, mybir.dt.bfloat16

    xt = nc.dram_tensor("xt", [D, T], bf, kind="ExternalInput").ap()
    wqt = nc.dram_tensor("wqt", [D, E], bf, kind="ExternalInput").ap()
    wkt = nc.dram_tensor("wkt", [D, HD], bf, kind="ExternalInput").ap()
    wvt = nc.dram_tensor("wvt", [D, HD], bf, kind="ExternalInput").ap()
    wot = nc.dram_tensor("wot", [D, D], bf, kind="ExternalInput").ap()
    csa = nc.dram_tensor("csa", [128, T], bf, kind="ExternalInput").ap()
    csb = nc.dram_tensor("csb", [128, T], bf, kind="ExternalInput").ap()
    mtl = nc.dram_tensor("mtl", [n_mtiles, 128, 2, 512], f32,
                         kind="ExternalInput").ap()
    out = nc.dram_tensor("out", [TSH, D], f32, kind="ExternalOutput").ap()

    with tile.TileContext(nc) as tc:
        _emit(nc, tc, plan, xt, wqt, wkt, wvt, wot, csa, csb, mtl, out,
              no_collective=no_collective)
    nc.compile()
    return nc


def _emit(nc, tc, plan, xt, wqt, wkt, wvt, wot, csa, csb, mtl, out,
          no_collective=False):
    f32, bf = mybir.dt.float32, mybir.dt.bfloat16
    Exp = mybir.ActivationFunctionType.Exp

    xt_r = xt.rearrange("(c p) t -> p c t", p=128)
    wqt_r = wqt.rearrange("(c p) e -> p c e", p=128)
    wkt_r = wkt.rearrange("(c p) e -> p c e", p=128)
    wvt_r = wvt.rearrange("(c p) e -> p c e", p=128)
    wot_r = wot.rearrange("(c p) o -> p c o", p=128)

    with tc.tile_pool(name="dram", bufs=1, space="DRAM") as dram:
        a2a_in = dram.tile([NCORES, E, TSH], bf)
        a2a_out = dram.tile([NCORES, E, TSH], bf)

        with tc.tile_pool(name="singles", bufs=1) as singles, \
             tc.tile_pool(name="xts", bufs=2) as xts, \
             tc.tile_pool(name="qkv", bufs=1) as qkv, \
             tc.tile_pool(name="ptp", bufs=2) as ptp, \
             tc.tile_pool(name="rp", bufs=3) as rp, \
             tc.tile_pool(name="small", bufs=4) as small, \
             tc.tile_pool(name="mgp", bufs=3) as mgp, \
             tc.tile_pool(name="mkp", bufs=2) as mkp, \
             tc.tile_pool(name="ps", bufs=2, space="PSUM") as ps, \
             tc.tile_pool(name="ps_s", bufs=2, space="PSUM") as ps_s, \
             tc.tile_pool(name="ps_o", bufs=1, space="PSUM") as ps_o, \
             tc.tile_pool(name="ps_d", bufs=1, space="PSUM") as ps_d, \
             tc.tile_pool(name="drs", bufs=4, space="DRAM") as drs:

            # --- resident weights / tables ---
            wq_sb = singles.tile([128, DCH, E], bf)
            nc.sync.dma_start(out=wq_sb, in_=wqt_r)
            wk_sb = singles.tile([128, DCH, HD], bf)
            nc.sync.dma_start(out=wk_sb, in_=wkt_r)
            wv_sb = singles.tile([128, DCH, HD], bf)
            nc.sync.dma_start(out=wv_sb, in_=wvt_r)
            csa_sb = singles.tile([128, T], bf)
            nc.sync.dma_start(out=csa_sb, in_=csa)
            csb_sb = singles.tile([128, T], bf)
            nc.sync.dma_start(out=csb_sb, in_=csb)
            ones_sb = singles.tile([128, 1], bf)
            nc.vector.memset(ones_sb, 1.0)

            def apply_rope(ps_tile, dst, t0):
                """dst = RoPE(ps_tile) using csa/csb tables at t0."""
                qsw = rp.tile([128, 512], bf, tag="qsw")
                nc.vector.tensor_copy(qsw[0:64, :], ps_tile[64:128, :])
                nc.vector.tensor_copy(qsw[64:128, :], ps_tile[0:64, :])
                ta = rp.tile([128, 512], bf, tag="ta")
                nc.vector.tensor_mul(ta, ps_tile, csa_sb[:, t0:t0 + 512])
                tb = rp.tile([128, 512], bf, tag="tb")
                nc.vector.tensor_mul(tb, qsw, csb_sb[:, t0:t0 + 512])
                nc.vector.tensor_add(dst, ta, tb)

            for b in range(B):
                # per-batch resident Q^T/K^T/V (rotated, bf16)
                q_t = [qkv.tile([128, S], bf, tag=f"q{h}", name=f"qt{h}")
                       for h in range(HLOC)]
                k_t = qkv.tile([128, S], bf, tag="k")
                v_t = qkv.tile([128, KC, HD], bf, tag="v")

                for tt in range(QT):  # t-tile within this batch (512 tokens)
                    t0 = b * S + tt * 512
                    # load x^T tile in two halves
                    xh = [xts.tile([128, DCH // 2, 512], bf, tag="x", name=f"xh{i}")
                          for i in range(2)]
                    for i2 in range(2):
                        nc.sync.dma_start(
                            out=xh[i2],
                            in_=xt_r[:, i2 * (DCH // 2):(i2 + 1) * (DCH // 2),
                                     t0:t0 + 512])

                    def xsl(ci):
                        return xh[ci // (DCH // 2)][:, ci % (DCH // 2), :]

                    # Q projections + RoPE
                    for h in range(HLOC):
                        q_ps = ps.tile([128, 512], f32, tag="proj")
                        for ci in range(DCH):
                            nc.tensor.matmul(
                                q_ps, wq_sb[:, ci, h * HD:(h + 1) * HD],
                                xsl(ci), start=(ci == 0), stop=(ci == DCH - 1))
                        apply_rope(q_ps, q_t[h][:, tt * 512:(tt + 1) * 512], t0)
                    # K projection + RoPE
                    k_ps = ps.tile([128, 512], f32, tag="proj")
                    for ci in range(DCH):
                        nc.tensor.matmul(k_ps, wk_sb[:, ci, :], xsl(ci),
                                         start=(ci == 0), stop=(ci == DCH - 1))
                    apply_rope(k_ps, k_t[:, tt * 512:(tt + 1) * 512], t0)
                    # V projection (token-major)
                    v_ps = ps.tile([128, 512], f32, tag="proj")
                    for j in range(4):
                        for ci in range(DCH):
                            nc.tensor.matmul(
                                v_ps[:, j * HD:(j + 1) * HD],
                                xsl(ci)[:, j * 128:(j + 1) * 128],
                                wv_sb[:, ci, :],
                                start=(ci == 0), stop=(ci == DCH - 1))
                    nc.scalar.copy(
                        v_t[:, tt * 4:(tt + 1) * 4, :].rearrange(
                            "p c e -> p (c e)"),
                        v_ps)

                    # --- attention for q-tile `tt`, all local heads ---
                    groups = plan[tt]
                    ng = len(groups)
                    for h in range(HLOC):
                        pt = ptp.tile([128, KC, 512], bf, tag="pt")
                        o_ps = ps_o.tile([128, 512], f32, tag="o")
                        den_ps = ps_d.tile([1, 512], f32, tag="den")
                        for gi, (kg, mode, mi) in enumerate(groups):
                            s_ps = ps_s.tile([128, 2, 512], f32, tag="s")
                            for i2 in range(2):
                                kcc = 2 * kg + i2
                                nc.tensor.matmul(
                                    s_ps[:, i2, :],
                                    k_t[:, kcc * 128:(kcc + 1) * 128],
                                    q_t[h][:, tt * 512:(tt + 1) * 512],
                                    start=True, stop=True)
                            if mode == MIXED_DATA:
                                mk = mkp.tile([128, 2, 512], f32, tag="mk")
                                nc.sync.dma_start(out=mk, in_=mtl[mi])
                                nc.vector.tensor_add(
                                    s_ps.rearrange("p a b -> p (a b)"),
                                    s_ps.rearrange("p a b -> p (a b)"),
                                    mk.rearrange("p a b -> p (a b)"))
                            pslice = pt[:, 2 * kg:2 * kg + 2, :]
                            nc.scalar.activation(
                                pslice.rearrange("p a b -> p (a b)"),
                                s_ps.rearrange("p a b -> p (a b)"),
                                Exp, scale=SCALE)
                            if mode == MIXED_CAUSAL:
                                # keep iff q - k >= 0 else 0
                                nc.gpsimd.affine_select(
                                    out=pslice, in_=pslice,
                                    pattern=[[-128, 2], [1, 512]],
                                    compare_op=mybir.AluOpType.is_ge,
                                    fill=0.0,
                                    base=tt * 512 - 256 * kg,
                                    channel_multiplier=-1)
                            for i2 in range(2):
                                kcc = 2 * kg + i2
                                first = gi == 0 and i2 == 0
                                last = gi == ng - 1 and i2 == 1
                                nc.tensor.matmul(o_ps, v_t[:, kcc, :],
                                                 pt[:, kcc, :],
                                                 start=first, stop=last,
                                                 skip_group_check=True)
                                nc.tensor.matmul(den_ps, ones_sb,
                                                 pt[:, kcc, :],
                                                 start=first, stop=last,
                                                 skip_group_check=True)
                        # evict O^T to SBUF right away so the PSUM bank frees
                        # for the next head's AV while the denominator's
                        # DRAM-roundtrip broadcast is still in flight
                        o_sb = mgp.tile([128, 512], f32, tag="osb")
                        nc.scalar.copy(o_sb, o_ps)
                        rec = small.tile([1, 512], f32, tag="rec")
                        nc.vector.reciprocal(rec, den_ps)
                        scr = drs.tile([1, 512], f32, tag="scr")
                        nc.sync.dma_start(out=scr[:], in_=rec)
                        bc = small.tile([128, 512], f32, tag="bc")
                        nc.sync.dma_start(
                            out=bc,
                            in_=bass.AP(tensor=scr.tensor, offset=scr.offset,
                                        ap=[[0, 128]] + scr.ap[1:]))
                        mg = mgp.tile([128, 512], bf, tag="mg")
                        nc.vector.tensor_mul(mg, o_sb, bc)
                        nc.sync.dma_start(
                            out=a2a_in[b * QT + tt, h * HD:(h + 1) * HD, :],
                            in_=mg)

        # --- AllToAll reshard: head-parallel -> token-parallel ---
        if no_collective:
            nc.sync.dma_start(out=a2a_out[:], in_=a2a_in[:])
        else:
            nc.gpsimd.collective_compute(
                "AllToAll", mybir.AluOpType.bypass,
                replica_groups=[list(range(NCORES))],
                ins=[a2a_in.opt()], outs=[a2a_out.opt()])

        # --- output projection for this core's 512-token shard ---
        with tc.tile_pool(name="mcp", bufs=1) as mcp, \
             tc.tile_pool(name="wop", bufs=2) as wop, \
             tc.tile_pool(name="outp", bufs=3) as outp, \
             tc.tile_pool(name="ps_c", bufs=3, space="PSUM") as ps_c:
            m_sb = mcp.tile([128, DCH, TSH], bf)
            nc.sync.dma_start(
                out=m_sb,
                in_=a2a_out.rearrange("r e t -> (r e) t").rearrange(
                    "(c p) t -> p c t", p=128))
            for ot in range(8):
                wo_sb = wop.tile([128, DCH, 512], bf, tag="wo")
                nc.sync.dma_start(out=wo_sb,
                                  in_=wot_r[:, :, ot * 512:(ot + 1) * 512])
                for tc4 in range(4):
                    o2 = ps_c.tile([128, 512], f32, tag="o2")
                    for ec in range(DCH):
                        nc.tensor.matmul(
                            o2, m_sb[:, ec, tc4 * 128:(tc4 + 1) * 128],
                            wo_sb[:, ec, :], start=(ec == 0),
                            stop=(ec == DCH - 1))
                    ot_sb = outp.tile([128, 512], f32, tag="outsb")
                    nc.scalar.copy(ot_sb, o2)
                    nc.sync.dma_start(
                        out=out[tc4 * 128:(tc4 + 1) * 128,
                                ot * 512:(ot + 1) * 512],
                        in_=ot_sb)


_CACHE = {}


def _get_nc(plan_key, plan, n_mtiles):
    if plan_key not in _CACHE:
        _CACHE[plan_key] = _build(plan, n_mtiles)
    return _CACHE[plan_key]


def _prep_inputs(x, freqs_cis, mask, wq, wk, wv, wo):
    x = np.asarray(x, np.float32).reshape(T, D)
    wq = np.asarray(wq, np.float32)
    wk = np.asarray(wk, np.float32)
    wv = np.asarray(wv, np.float32)
    wo = np.asarray(wo, np.float32)
    fc = np.asarray(freqs_cis, np.float32)

    xt = np.ascontiguousarray(x.T).astype(BF16)            # (D, T)
    permq = _pair_split_perm(N_HEADS)
    permk = _pair_split_perm(N_KV)
    wq_p = wq[permq]
    wk_p = wk[permk]
    wot = np.ascontiguousarray(wo.T).astype(BF16)          # (D, D)

    cos = fc[:, :, 0].T                                    # (64, S)
    sin = fc[:, :, 1].T
    cos2 = np.concatenate([cos, cos], axis=1)              # (64, T) both batches
    sin2 = np.concatenate([sin, sin], axis=1)
    csa = np.concatenate([cos2, cos2], axis=0).astype(BF16)   # (128, T)
    csb = np.concatenate([-sin2, sin2], axis=0).astype(BF16)

    plan, mtiles = _classify_mask(mask)

    in_maps = []
    for c in range(NCORES):
        wqt_c = np.ascontiguousarray(wq_p[c * E:(c + 1) * E].T).astype(BF16)
        wkt_c = np.ascontiguousarray(wk_p[c * HD:(c + 1) * HD].T).astype(BF16)
        wvt_c = np.ascontiguousarray(wv[c * HD:(c + 1) * HD].T).astype(BF16)
        in_maps.append({
            "xt": xt, "wqt": wqt_c, "wkt": wkt_c, "wvt": wvt_c,
            "wot": wot, "csa": csa, "csb": csb, "mtl": mtiles,
        })
    return in_maps, plan, mtiles


def kernel(x, freqs_cis, mask, wq, wk, wv, wo, start_pos=0, **_unused):
    in_maps, plan, mtiles = _prep_inputs(x, freqs_cis, mask, wq, wk, wv, wo)
    plan_key = (tuple(tuple(r) for r in plan), mtiles.shape[0])
    nc = _get_nc(plan_key, plan, mtiles.shape[0])
    last_err = None
    for _attempt in range(3):
        try:
            res = run_bass_kernel_spmd(nc, in_maps,
                                       core_ids=list(range(NCORES)))
            break
        except Exception as e:  # transient NRT device errors — retry
            last_err = e
    else:
        raise last_err
    shards = [res.results[c]["out"] for c in range(NCORES)]
    full = np.concatenate(shards, axis=0)                  # (T, D)
    return full.reshape(B, S, D).astype(np.float32)
